# revision 1
# baseline (speedup 1.0000x reference)
"""AVWDCRNN kernel for 8 Trainium2 NeuronCores.

Strategy (per sharding_hint): data-parallel over batch B=32 across the 8
cores (B_local=4 per core); all parameters replicated; the sequential scan
over T=12 stays local per device. Inputs are taken FULL and sharded here;
output is gathered back to full shape.

Algebraic optimizations applied before launch (host-side, exact):
  * per-node adaptive weights are fused with the post-concat linear layer:
      W2[n] = (emb @ wpool)[n] @ cW[:, :O].T          [N, 66, O]
      bias2[n] = (emb @ bpool)[n] @ cW[:, :O].T + emb[n] @ cW[:, O:].T + cb
  * the temporal hyperedge term inside the cell collapses to a scalar
    (T=1 inside the scan): tp = sum(ht^2) * xc
so each scan step is: xc = per-node-linear(in) ; zr/hc =
  act(gam*((1+c)*xc + Hs @ (Hs.T @ xc)) + bet).
"""

import numpy as np

B, T, N, DIN, DOUT, D, SK, TK = 32, 12, 1024, 2, 64, 10, 16, 4
NCORES = 8
BL = B // NCORES  # 4 per core

_COMPILED = {}


def _build():
    import jax
    import jax.numpy as jnp

    HI = jax.lax.Precision.HIGHEST

    def hyper_layer(x, Hs, Ht, gamma, beta):
        # x [b,T,N,C] full layer-level hypernet (temporal dim real here)
        es = jnp.einsum('btnc,ns->btsc', x, Hs, precision=HI)
        sp = jnp.einsum('ms,btsc->btmc', Hs, es, precision=HI)
        et = jnp.einsum('btnc,tk->bknc', x, Ht, precision=HI)
        tp = jnp.einsum('uk,bknc->bunc', Ht, et, precision=HI)
        return gamma * (x + sp + tp) + beta

    def cell_gcn(inp, W2, bias2, Hs, cscal, gamma, beta):
        # inp [b,N,I]; W2 [N,I,O]; bias2 [N,O]
        xc = jnp.einsum('bni,nio->bno', inp, W2, precision=HI) + bias2
        es = jnp.einsum('bnc,ns->bsc', xc, Hs, precision=HI)
        sp = jnp.einsum('ms,bsc->bmc', Hs, es, precision=HI)
        return gamma * ((1.0 + cscal) * xc + sp) + beta

    def shard_fn(x, h0, W2g, b2g, hsg, cg, gamg, betg,
                 W2u, b2u, hsu, cu, gamu, betu, lhs, lht, lgam, lbet):
        # x [BL,T,N,DIN], h0 [BL,N,DOUT]
        def step(state, x_t):
            zr = jax.nn.sigmoid(cell_gcn(
                jnp.concatenate([x_t, state], -1), W2g, b2g, hsg, cg, gamg, betg))
            z, r = zr[..., :DOUT], zr[..., DOUT:]
            hc = jnp.tanh(cell_gcn(
                jnp.concatenate([x_t, z * state], -1), W2u, b2u, hsu, cu, gamu, betu))
            h = r * state + (1.0 - r) * hc
            return h, h

        last, seq = jax.lax.scan(step, h0, jnp.swapaxes(x, 0, 1))
        seq = jnp.swapaxes(seq, 0, 1)  # [BL,T,N,DOUT]
        out = hyper_layer(seq, lhs, lht, lgam, lbet)
        return out, last

    pfn = jax.pmap(shard_fn, axis_name='b',
                   in_axes=(0, 0) + (None,) * 16)
    return pfn


def kernel(x, init_state, node_embeddings, wpool_g, bpool_g, cW_g, cb_g,
           hs_g, ht_g, gam_g, bet_g, wpool_u, bpool_u, cW_u, cb_u,
           hs_u, ht_u, gam_u, bet_u, lhs, lht, lgam, lbet):
    import jax.numpy as jnp

    f32 = np.float32
    x = np.asarray(x, f32)
    init_state = np.asarray(init_state, f32)
    emb = np.asarray(node_embeddings, f32)

    def fuse(wpool, bpool, cW, cb, O):
        w = np.einsum('nd,dio->nio', emb, np.asarray(wpool, f32))      # [N,66,O]
        b = emb @ np.asarray(bpool, f32)                                # [N,O]
        cW = np.asarray(cW, f32)
        cWx, cWe = cW[:, :O], cW[:, O:]
        W2 = np.einsum('nio,po->nip', w, cWx)                           # [N,66,O]
        bias2 = b @ cWx.T + emb @ cWe.T + np.asarray(cb, f32)           # [N,O]
        return W2, bias2

    W2g, b2g = fuse(wpool_g, bpool_g, cW_g, cb_g, 2 * DOUT)
    W2u, b2u = fuse(wpool_u, bpool_u, cW_u, cb_u, DOUT)
    cg = f32(np.sum(np.asarray(ht_g, f32) ** 2))
    cu = f32(np.sum(np.asarray(ht_u, f32) ** 2))

    if 'pfn' not in _COMPILED:
        _COMPILED['pfn'] = _build()
    pfn = _COMPILED['pfn']

    xs = x.reshape(NCORES, BL, T, N, DIN)
    h0 = init_state[0].reshape(NCORES, BL, N, DOUT)

    out, last = pfn(xs, h0, W2g, b2g, np.asarray(hs_g, f32), cg,
                    np.asarray(gam_g, f32), np.asarray(bet_g, f32),
                    W2u, b2u, np.asarray(hs_u, f32), cu,
                    np.asarray(gam_u, f32), np.asarray(bet_u, f32),
                    np.asarray(lhs, f32), np.asarray(lht, f32),
                    np.asarray(lgam, f32), np.asarray(lbet, f32))

    out = np.asarray(out).reshape(B, T, N, DOUT)
    last = np.asarray(last).reshape(B, N, DOUT)
    return out, last


# revision 6
# speedup vs baseline: 1.0328x; 1.0328x over previous
"""AVWDCRNN kernel for 8 Trainium2 NeuronCores.

Strategy (per sharding_hint): data-parallel over batch B=32 across the 8
cores (B_local=4 per core); all parameters replicated; the sequential scan
over T=12 stays local per device. Inputs are taken FULL and sharded here;
output is gathered back to full shape.

Algebraic optimizations applied before launch (host-side, exact):
  * per-node adaptive weights are fused with the post-concat linear layer:
      W2[n] = (emb @ wpool)[n] @ cW[:, :O].T          [N, 66, O]
      bias2[n] = (emb @ bpool)[n] @ cW[:, :O].T + emb[n] @ cW[:, O:].T + cb
  * the temporal hyperedge term inside the cell collapses to a scalar
    (T=1 inside the scan): tp = sum(ht^2) * xc
so each scan step is: xc = per-node-linear(in) ; zr/hc =
  act(gam*((1+c)*xc + Hs @ (Hs.T @ xc)) + bet).
"""

import numpy as np

B, T, N, DIN, DOUT, D, SK, TK = 32, 12, 1024, 2, 64, 10, 16, 4
NCORES = 8
BL = B // NCORES  # 4 per core

_COMPILED = {}
import os
KR_MODE = os.environ.get('KR_MODE', 'pernode')


def _build():
    import jax
    import jax.numpy as jnp

    HI = jax.lax.Precision.HIGHEST

    def hyper_layer(x, Hs, Ht, gamma, beta):
        # x [b,T,N,C] full layer-level hypernet (temporal dim real here)
        es = jnp.einsum('btnc,ns->btsc', x, Hs, precision=HI)
        sp = jnp.einsum('ms,btsc->btmc', Hs, es, precision=HI)
        et = jnp.einsum('btnc,tk->bknc', x, Ht, precision=HI)
        tp = jnp.einsum('uk,bknc->bunc', Ht, et, precision=HI)
        return gamma * (x + sp + tp) + beta

    def cell_gcn(inp, emb, Wbig, bias2, Hs, cscal, gamma, beta):
        # inp [b,N,I]; Wbig [D*I,O]; bias2 [N,O].  Khatri-Rao expansion of the
        # per-node adaptive matmul into one shared dense matmul.
        bl, n, i = inp.shape
        if Wbig.ndim == 3:  # host-precomputed per-node weights [N,I,O]
            xc = jnp.einsum('bni,nio->bno', inp, Wbig, precision=HI) + bias2
        else:
            xd = (emb[None, :, :, None] * inp[:, :, None, :]).reshape(bl * n, D * i)
            xc = jnp.dot(xd, Wbig, precision=HI).reshape(bl, n, -1) + bias2
        es = jnp.einsum('bnc,ns->bsc', xc, Hs, precision=HI)
        sp = jnp.einsum('ms,bsc->bmc', Hs, es, precision=HI)
        return gamma * ((1.0 + cscal) * xc + sp) + beta

    def shard_fn(x, h0, emb, Wbg, b2g, hsg, cg, gamg, betg,
                 Wbu, b2u, hsu, cu, gamu, betu, lhs, lht, lgam, lbet):
        # x [BL,T,N,DIN], h0 [BL,N,DOUT]
        def step(state, x_t):
            zr = jax.nn.sigmoid(cell_gcn(
                jnp.concatenate([x_t, state], -1), emb, Wbg, b2g, hsg, cg, gamg, betg))
            z, r = zr[..., :DOUT], zr[..., DOUT:]
            hc = jnp.tanh(cell_gcn(
                jnp.concatenate([x_t, z * state], -1), emb, Wbu, b2u, hsu, cu, gamu, betu))
            h = r * state + (1.0 - r) * hc
            return h, h

        last, seq = jax.lax.scan(step, h0, jnp.swapaxes(x, 0, 1))
        seq = jnp.swapaxes(seq, 0, 1)  # [BL,T,N,DOUT]
        out = hyper_layer(seq, lhs, lht, lgam, lbet)
        return out, last

    pfn = jax.pmap(shard_fn, axis_name='b',
                   in_axes=(0, 0) + (None,) * 17)
    return pfn


def kernel(x, init_state, node_embeddings, wpool_g, bpool_g, cW_g, cb_g,
           hs_g, ht_g, gam_g, bet_g, wpool_u, bpool_u, cW_u, cb_u,
           hs_u, ht_u, gam_u, bet_u, lhs, lht, lgam, lbet):
    import jax.numpy as jnp

    f32 = np.float32
    x = np.asarray(x, f32)
    init_state = np.asarray(init_state, f32)
    emb = np.asarray(node_embeddings, f32)

    def fuse(wpool, bpool, cW, cb, O):
        wpool = np.asarray(wpool, f32)                                  # [D,66,O]
        b = emb @ np.asarray(bpool, f32)                                # [N,O]
        cW = np.asarray(cW, f32)
        cWx, cWe = cW[:, :O], cW[:, O:]
        # Wbig[(d,i),o] = (wpool[d] @ cWx.T)[i,o]  (fused with output linear)
        Wbig = np.einsum('dio,po->dip', wpool, cWx)                     # [D,I,O]
        bias2 = b @ cWx.T + emb @ cWe.T + np.asarray(cb, f32)           # [N,O]
        if KR_MODE == 'pernode':
            W2 = np.einsum('nd,dio->nio', emb, Wbig)                    # [N,I,O]
            return np.ascontiguousarray(W2), bias2
        return np.ascontiguousarray(Wbig.reshape(-1, O)), bias2

    Wbg, b2g = fuse(wpool_g, bpool_g, cW_g, cb_g, 2 * DOUT)
    Wbu, b2u = fuse(wpool_u, bpool_u, cW_u, cb_u, DOUT)
    cg = f32(np.sum(np.asarray(ht_g, f32) ** 2))
    cu = f32(np.sum(np.asarray(ht_u, f32) ** 2))

    if 'pfn' not in _COMPILED:
        _COMPILED['pfn'] = _build()
    pfn = _COMPILED['pfn']

    xs = x.reshape(NCORES, BL, T, N, DIN)
    h0 = init_state[0].reshape(NCORES, BL, N, DOUT)

    out, last = pfn(xs, h0, emb, Wbg, b2g, np.asarray(hs_g, f32), cg,
                    np.asarray(gam_g, f32), np.asarray(bet_g, f32),
                    Wbu, b2u, np.asarray(hs_u, f32), cu,
                    np.asarray(gam_u, f32), np.asarray(bet_u, f32),
                    np.asarray(lhs, f32), np.asarray(lht, f32),
                    np.asarray(lgam, f32), np.asarray(lbet, f32))

    out = np.asarray(out).reshape(B, T, N, DOUT)
    last = np.asarray(last).reshape(B, N, DOUT)
    return out, last
